# revision 5
# baseline (speedup 1.0000x reference)
"""Trainium2 Bass kernel for nn_Attention_68186900791341 (sparse_attention).

Math (per batch b of 4096, N=200 tokens, K=64):
    emb = [q, k, q-k, q*k]  ->  emb @ W1 == k @ (W1b-W1c) + (q*k) @ W1d + q @ (W1a+W1c)
Host folds q into per-batch L1 weights:
    W1AUG[b] = (W1b - W1c) + q[b][:,None] * W1d        [64, 80]  (bf16)
    C[b]     = q[b] @ (W1a + W1c) + b1                 [80]      (fp32, ACT Prelu bias)
Device (per core, 512 batches; pure data parallel across 8 cores):
  - k_T via PE transposes (bf16), h1 = Prelu(W1AUG_b.T @ k_T + C_b) on ACT
    (per-channel alpha), h2 = Prelu(W2.T @ h1 + b2), score = Wo.T @ h2 with
    tile_position col-groups packing 4 pairs (of 2 batches) per PSUM tile.
  - Fixed-shift softmax, no max-reduce: probs = exp(score - 40) straight from
    PSUM (scores empirically in [-3, 4]); masking is applied MULTIPLICATIVELY
    after transposing probs to column layout (host-prepared bf16 mask in the
    same column layout). Unnormalized weights contract with [V | 1] so the
    softmax denominator falls out of the matmul's ones-column.
  - All cross-partition data movement via PE transposes + strided-FREE-dim
    copies (partition-strided APs are illegal / broken in this toolchain).
"""
import sys
sys.path.insert(0, "/opt/trn_rl_repo")
import numpy as np
import ml_dtypes

import concourse.bass as bass
import concourse.tile as tile
from concourse import mybir
from concourse.bass_utils import run_bass_kernel_spmd
from concourse.masks import make_identity

dt = mybir.dt

N_CORES = 8
B, N, K = 4096, 200, 64
H1, H2 = 80, 40
BC = B // N_CORES          # 512 batches per core
NS = BC // 16              # 32 supertiles of 16 batches
NB = BC // 64              # 8 blocks of 64 batches
P = 100                    # token-partition layout: 200 tokens = 2 chunks of 100
SHIFT = -40.0              # fixed softmax shift

_CACHE = {}


def split_waits(nc):
    """This walrus build allows only ONE sync wait per instruction. Hoist
    excess waits onto same-engine InstEventSemaphore carriers placed before."""
    import bass_rust as _b
    for f in nc.m.functions:
        for blk in f.blocks:
            insts = list(blk.instructions)
            out, changed = [], False
            for inst in insts:
                si = inst.sync_info
                nw = len(si.on_wait) if si else 0
                if nw >= 2:
                    for j, w in enumerate(list(si.on_wait)[:-1]):
                        out.append(_b.InstEventSemaphore(
                            name=f"{inst.name}-wc{j}", engine=inst.engine,
                            ins=[], outs=[],
                            sync_info=_b.SyncInfo(on_wait=[w], on_update=[]),
                        ))
                    inst.sync_info = _b.SyncInfo(
                        on_wait=[list(si.on_wait)[-1]], on_update=list(si.on_update))
                    changed = True
                out.append(inst)
            if changed:
                blk.instructions = out


def _free_stride(ap, step, n):
    """View of a 2D AP taking every `step`-th element of the last (free) dim."""
    return bass.AP(ap.tensor, ap.offset, list(ap.ap[:-1]) + [[step, n]])


def build():
    nc = bass.Bass()
    key = nc.dram_tensor("key", [BC * N, K], dt.float32, kind="ExternalInput")
    val = nc.dram_tensor("val", [BC * N, K], dt.float32, kind="ExternalInput")
    w1aug = nc.dram_tensor("w1aug", [NS, K, 16, H1], dt.bfloat16, kind="ExternalInput")
    ctab = nc.dram_tensor("ctab", [H1, BC], dt.float32, kind="ExternalInput")
    mcols = nc.dram_tensor("mcols", [NB, P, 8, 4, 4], dt.bfloat16, kind="ExternalInput")
    w2 = nc.dram_tensor("w2", [H1, H2], dt.bfloat16, kind="ExternalInput")
    wo = nc.dram_tensor("wo", [H2, 1], dt.bfloat16, kind="ExternalInput")
    a1t = nc.dram_tensor("a1t", [H1, 1], dt.float32, kind="ExternalInput")
    a2t = nc.dram_tensor("a2t", [H2, 1], dt.float32, kind="ExternalInput")
    b2t = nc.dram_tensor("b2t", [H2, 1], dt.float32, kind="ExternalInput")
    out = nc.dram_tensor("out", [BC, K], dt.float32, kind="ExternalOutput")

    kview = key.rearrange("(s a p) k -> s p a k", s=NS, p=P)   # [NS, 100, 32, 64]
    vview = val.rearrange("(s a p) k -> s p a k", s=NS, p=P)

    with tile.TileContext(nc) as tc:
        with (
            tc.tile_pool(name="loads", bufs=2) as loads,
            tc.tile_pool(name="vpool", bufs=5) as vpool,
            tc.tile_pool(name="work", bufs=2) as work,
            tc.tile_pool(name="hh", bufs=3) as hh,
            tc.tile_pool(name="psT", bufs=2, space="PSUM") as psT,      # k-transpose
            tc.tile_pool(name="psH1", bufs=2, space="PSUM") as psH1,
            tc.tile_pool(name="psH2", bufs=1, space="PSUM") as psH2,
            tc.tile_pool(name="psSC", bufs=1, space="PSUM") as psSC,    # score groups
            tc.tile_pool(name="psS", bufs=1, space="PSUM") as psS,      # small transposes
            tc.tile_pool(name="psV", bufs=1, space="PSUM") as psV,
            tc.tile_pool(name="singles", bufs=1) as singles,
        ):
            ident = singles.tile([128, 128], dt.bfloat16)
            make_identity(nc, ident)
            identf = singles.tile([128, 128], dt.float32)
            make_identity(nc, identf)
            w2_sb = singles.tile([H1, H2], dt.bfloat16)
            nc.sync.dma_start(w2_sb, w2[:, :])
            wo_sb = singles.tile([H2, 1], dt.bfloat16)
            nc.sync.dma_start(wo_sb, wo[:, :])
            a1_sb = singles.tile([H1, 1], dt.float32)
            nc.sync.dma_start(a1_sb, a1t[:, :])
            a2_sb = singles.tile([H2, 1], dt.float32)
            nc.sync.dma_start(a2_sb, a2t[:, :])
            b2_sb = singles.tile([H2, 1], dt.float32)
            nc.sync.dma_start(b2_sb, b2t[:, :])
            ct_sb = singles.tile([H1, BC], dt.float32)
            nc.sync.dma_start(ct_sb, ctab[:, :])
            negC = singles.tile([128, 1], dt.float32)
            nc.vector.memset(negC, SHIFT)

            for blk_i in range(NB):
                mc_sb = loads.tile([P, 8, 4, 4], dt.bfloat16, tag="mc")
                nc.sync.dma_start(mc_sb, mcols[blk_i])
                outT = work.tile([K + 1, 64], dt.float32, tag="outT")

                for s_loc in range(4):
                    s = blk_i * 4 + s_loc
                    k_f32 = loads.tile([P, 32, K], dt.float32, tag="kf")
                    nc.sync.dma_start(k_f32, kview[s])
                    v_f32 = loads.tile([P, 32, K], dt.float32, tag="vf")
                    nc.sync.dma_start(v_f32, vview[s])
                    w1_sb = loads.tile([K, 16, H1], dt.bfloat16, tag="w1")
                    nc.sync.dma_start(w1_sb, w1aug[s])

                    k_bf = work.tile([P, 32, K], dt.bfloat16, tag="kbf")
                    nc.gpsimd.tensor_copy(k_bf, k_f32)
                    v_bf = vpool.tile([P, 32, K + 1], dt.bfloat16, tag="vbf")
                    nc.gpsimd.tensor_copy(v_bf[:, :, 0:K], v_f32)
                    nc.gpsimd.memset(v_bf[:, :, K:K + 1], 1.0)

                    kT = work.tile([K, 3200], dt.bfloat16, tag="kT")
                    for a0 in range(0, 32, 8):
                        tp = psT.tile([K, 8, P], dt.bfloat16, tag="tp")
                        for j in range(8):
                            nc.tensor.transpose(tp[:, j], k_bf[:, a0 + j],
                                                ident[:P, :P])
                        nc.vector.tensor_copy(
                            kT[:, a0 * P:(a0 + 8) * P],
                            tp.rearrange("p a f -> p (a f)"))

                    for half in range(2):
                        grp = 2 * s_loc + half
                        score_ps = psSC.tile([128, 400], dt.float32, tag="sc")
                        for j in range(4):
                            g = 4 * half + j          # pair within supertile
                            h1_ps = psH1.tile([H1, 400], dt.float32, tag="h1ps")
                            for t in range(2):
                                bl = 2 * g + t
                                nc.tensor.matmul(
                                    h1_ps[:, t * 200:(t + 1) * 200],
                                    w1_sb[:, bl], kT[:, bl * 200:bl * 200 + 200],
                                    start=True, stop=True)
                            h1 = hh.tile([H1, 400], dt.bfloat16, tag="h1")
                            for t in range(2):
                                gb = s * 16 + 2 * g + t
                                nc.scalar.activation(
                                    h1[:, t * 200:(t + 1) * 200],
                                    h1_ps[:, t * 200:(t + 1) * 200],
                                    mybir.ActivationFunctionType.Prelu,
                                    bias=ct_sb[:, gb:gb + 1], scale=1.0,
                                    alpha=a1_sb[:, 0:1])
                            h2_ps = psH2.tile([H2, 400], dt.float32, tag="h2ps")
                            nc.tensor.matmul(h2_ps, w2_sb, h1, start=True, stop=True)
                            h2 = hh.tile([H2, 400], dt.bfloat16, tag="h2")
                            nc.scalar.activation(h2, h2_ps,
                                                 mybir.ActivationFunctionType.Prelu,
                                                 bias=b2_sb[:, 0:1], scale=1.0,
                                                 alpha=a2_sb[:, 0:1])
                            nc.tensor.matmul(score_ps[32 * j:32 * j + 1, :], wo_sb,
                                             h2, start=True, stop=True,
                                             tile_position=(0, 32 * j))

                        # ---- per 4-pair group: exp, transpose, mask, V ----
                        probs = work.tile([128, 400], dt.bfloat16, tag="probs")
                        nc.scalar.activation(probs, score_ps,
                                             mybir.ActivationFunctionType.Exp,
                                             bias=negC[:, 0:1], scale=1.0)
                        wcols = work.tile([P, 4, 4], dt.bfloat16, tag="wcols")
                        for c in range(4):
                            tpp = psS.tile([P, 128], dt.bfloat16, tag="sm")
                            nc.tensor.transpose(tpp, probs[:, c * P:(c + 1) * P],
                                                ident)
                            nc.vector.tensor_copy(wcols[:, c, :],
                                                  _free_stride(tpp[:, :], 32, 4))
                        nc.vector.tensor_tensor(wcols, wcols, mc_sb[:, grp],
                                                mybir.AluOpType.mult)

                        for q in range(2):
                            v_ps = psV.tile([128, K + 1], dt.float32, tag="vout")
                            for r in range(4):
                                bq = 4 * q + r         # batch within group (0..7)
                                j, t = bq // 2, bq % 2
                                u = 2 * (4 * half + j) + t   # batch within supertile
                                for h in range(2):
                                    nc.tensor.matmul(
                                        v_ps[32 * r:32 * r + 1, :],
                                        wcols[:, 2 * t + h, j:j + 1],
                                        v_bf[:, 2 * u + h, :],
                                        start=(h == 0), stop=(h == 1),
                                        tile_position=(0, 32 * r))
                            v_tmp = work.tile([128, K + 1], dt.float32, tag="vtmp")
                            nc.vector.tensor_copy(v_tmp, v_ps)
                            tpv = psS.tile([K + 1, 128], dt.float32, tag="sm")
                            nc.tensor.transpose(tpv, v_tmp, identf)
                            nc.vector.tensor_copy(
                                outT[:, 8 * grp + 4 * q: 8 * grp + 4 * q + 4],
                                _free_stride(tpv[:, :], 32, 4))

                # ---- block tail: final transpose + normalize + store ----
                outP = psS.tile([64, K + 1], dt.float32, tag="sm")
                nc.tensor.transpose(outP, outT, identf[:K + 1, :K + 1])
                recip = work.tile([64, 1], dt.float32, tag="recip")
                nc.vector.reciprocal(recip, outP[:, K:K + 1])
                out_sb = work.tile([64, K], dt.float32, tag="outsb")
                nc.vector.tensor_scalar(out_sb, outP[:, 0:K], recip[:, 0:1], None,
                                        mybir.AluOpType.mult)
                nc.sync.dma_start(out[blk_i * 64:(blk_i + 1) * 64, :], out_sb)

    split_waits(nc)
    return nc


def _prep_inputs(query, key, value, mask, W1, b1):
    """Host-side weight folding + mask layout (cheap numpy)."""
    W1a, W1b, W1c, W1d = W1[0:K], W1[K:2 * K], W1[2 * K:3 * K], W1[3 * K:4 * K]
    W1eff = (W1b - W1c)[None] + query[:, :, None] * W1d[None]     # [B, 64, 80]
    C = query @ (W1a + W1c) + b1                                   # [B, 80]
    w1aug_full = np.ascontiguousarray(
        W1eff.reshape(N_CORES, NS, 16, K, H1).transpose(0, 1, 3, 2, 4)
    ).astype(ml_dtypes.bfloat16)                                   # [8, NS, 64, 16, 80]
    ctab_full = np.ascontiguousarray(
        C.reshape(N_CORES, BC, H1).transpose(0, 2, 1)).astype(np.float32)
    # mask -> column layout: mcols[core, nb, p, grp, c, j] =
    #   mask[b0 + 2J + c//2, 100*(c%2) + p],  J = (grp//2)*8 + (grp%2)*4 + j
    m5 = mask.reshape(N_CORES, NB, 32, 2, 2, P)    # [core, nb, J, t, hchunk, p]
    Jidx = np.arange(32)
    grp_of_J = (Jidx // 8) * 2 + (Jidx % 8) // 4   # [32]
    j_of_J = Jidx % 4
    mcols_full = np.zeros((N_CORES, NB, P, 8, 4, 4), dtype=np.float32)
    for J in range(32):
        g, j = grp_of_J[J], j_of_J[J]
        for t in range(2):
            for h in range(2):
                mcols_full[:, :, :, g, 2 * t + h, j] = m5[:, :, J, t, h, :]
    mcols_full = mcols_full.astype(ml_dtypes.bfloat16)
    in_maps = []
    for i in range(N_CORES):
        sl = slice(i * BC, (i + 1) * BC)
        in_maps.append(dict(
            key=np.ascontiguousarray(key[sl]).reshape(BC * N, K),
            val=np.ascontiguousarray(value[sl]).reshape(BC * N, K),
            w1aug=w1aug_full[i],
            ctab=ctab_full[i],
            mcols=mcols_full[i],
        ))
    return in_maps


def kernel(query, key, value, mask, W1, b1, a1, W2, b2, a2, Wo, bo,
           _trace=False, _trace_kwargs=None):
    query = np.asarray(query, dtype=np.float32)
    key = np.asarray(key, dtype=np.float32)
    value = np.asarray(value, dtype=np.float32)
    mask = np.asarray(mask, dtype=np.int32)
    W1 = np.asarray(W1, dtype=np.float32)
    b1 = np.asarray(b1, dtype=np.float32)
    # bo is mathematically irrelevant (softmax shift-invariance)
    shared = dict(
        w2=np.asarray(W2, dtype=np.float32).astype(ml_dtypes.bfloat16),
        wo=np.asarray(Wo, dtype=np.float32).astype(ml_dtypes.bfloat16).reshape(H2, 1),
        a1t=np.asarray(a1, dtype=np.float32).reshape(H1, 1),
        a2t=np.asarray(a2, dtype=np.float32).reshape(H2, 1),
        b2t=np.asarray(b2, dtype=np.float32).reshape(H2, 1),
    )
    in_maps = _prep_inputs(query, key, value, mask, W1, b1)
    for m in in_maps:
        m.update(shared)
    if "nc" not in _CACHE:
        _CACHE["nc"] = build()
    res = run_bass_kernel_spmd(
        _CACHE["nc"], in_maps, list(range(N_CORES)),
        trace=_trace, **(_trace_kwargs or {}))
    outs = np.concatenate([res.results[i]["out"] for i in range(N_CORES)], axis=0)
    if _trace:
        return outs, res
    return outs
